# revision 10
# baseline (speedup 1.0000x reference)
"""Causal multi-head attention (B=1, S=4096, D=768, H=12, d_head=64) on 8
Trainium2 NeuronCores.

Sharding: tensor-parallel over heads. 12 heads are mapped onto 16 head-slots
(2 per core); the 4 leftover heads are duplicated onto two slots of the same
core with their W_out rows pre-scaled by 0.5, keeping the SPMD program
uniform across cores. Each core computes Q/K/V projections for its 2 head
slots, causal flash-attention (exp without max-subtraction; softmax
denominator obtained free via an appended ones-column on V), and a partial
row-parallel out-projection. The host sums the 8 partial outputs and adds
b_out (the all-reduce step of the row-parallel out projection).
"""

import sys

sys.path.insert(0, "/opt/trn_rl_repo")

import numpy as np

import concourse.bass as bass
import concourse.tile as tile
from concourse import bacc, mybir
from concourse.bass_utils import run_bass_kernel_spmd

S = 4096
D = 768
HD = 64
P = 128
KC = D // P  # 6 contraction chunks for the projections
QT_W = 512  # query-tile width (psum free dim)
NQT = S // QT_W  # 8 query tiles
NKB = S // P  # 32 key blocks
NEG = -1e30

F32 = mybir.dt.float32
F32R = mybir.dt.float32r

# head slots: cores 0-3 take head pairs (0,1)..(6,7); cores 4-7 take one of
# heads 8-11 in both slots with W_out halved.
SLOTS = [(0, 1), (2, 3), (4, 5), (6, 7), (8, 8), (9, 9), (10, 10), (11, 11)]
SCALES = [(1.0, 1.0)] * 4 + [(0.5, 0.5)] * 4

_CACHED_NC = None


def build_nc():
    nc = bacc.Bacc("TRN2", target_bir_lowering=False, debug=False, num_devices=8)

    x_d = nc.declare_dram_parameter("x", [S, D], F32, isOutput=False)
    wq_d = nc.declare_dram_parameter("wq", [D, P], F32, isOutput=False)
    wk_d = nc.declare_dram_parameter("wk", [D, P], F32, isOutput=False)
    wv_d = nc.declare_dram_parameter("wv", [D, P], F32, isOutput=False)
    wo_d = nc.declare_dram_parameter("wo", [P, D], F32, isOutput=False)
    mask_d = nc.declare_dram_parameter("mask", [P, P], F32, isOutput=False)
    ident_d = nc.declare_dram_parameter("ident", [P, P], F32, isOutput=False)
    out_d = nc.declare_dram_parameter("out", [S, D], F32, isOutput=True)

    with tile.TileContext(nc) as tc:
        with (
            tc.tile_pool(name="const", bufs=1) as const,
            tc.tile_pool(name="big", bufs=1) as big,
        ):
            # ---- constants ----
            mask_s = const.tile([P, P], F32)
            nc.sync.dma_start(mask_s[:], mask_d[:])
            ident = const.tile([P, P], F32)
            nc.sync.dma_start(ident[:], ident_d[:])
            ident_r = const.tile([P, P], F32R)
            nc.vector.tensor_copy(ident_r[:], ident[:])

            w_r = const.tile([P, KC, 3 * P], F32R)
            wo_r = const.tile([P, D], F32R)
            with tc.tile_pool(name="wst", bufs=1) as wst:
                w_stage = wst.tile([P, KC, 3 * P], F32)
                nc.sync.dma_start(
                    w_stage[:, :, 0:P], wq_d.rearrange("(c p) m -> p c m", p=P)
                )
                nc.sync.dma_start(
                    w_stage[:, :, P : 2 * P], wk_d.rearrange("(c p) m -> p c m", p=P)
                )
                nc.sync.dma_start(
                    w_stage[:, :, 2 * P : 3 * P],
                    wv_d.rearrange("(c p) m -> p c m", p=P),
                )
                nc.vector.tensor_copy(w_r[:], w_stage[:])
                wo_stage = wst.tile([P, D], F32)
                nc.sync.dma_start(wo_stage[:], wo_d[:])
                nc.vector.tensor_copy(wo_r[:], wo_stage[:])

            # ---- persistent big tensors ----
            xT = big.tile([P, KC, S], F32R)  # x transposed, d on partitions
            qT = big.tile([P, S], F32R)  # slot A rows 0:64, slot B rows 64:128
            kT = big.tile([P, S], F32R)
            vA = big.tile([P, NKB, 130], F32R)  # V natural +ones col per slot
            # normalized ctx.T reuses qT's storage: each qT query-slice is
            # dead once its q-tile's score matmuls are done.
            cT = qT

            # ---- phase 1: load x, transpose via PE ----
            with (
                tc.tile_pool(name="xs", bufs=3) as xs,
                tc.tile_pool(name="trp", bufs=4, space="PSUM") as trp,
            ):
                for st in range(S // P):
                    x_stage = xs.tile([P, D], F32)
                    nc.sync.dma_start(x_stage[:], x_d[st * P : (st + 1) * P, :])
                    for c in range(KC):
                        tp = trp.tile([P, P], F32)
                        nc.tensor.transpose(
                            tp[:], x_stage[:, c * P : (c + 1) * P], ident[:]
                        )
                        if (st + c) % 2 == 0:
                            nc.vector.tensor_copy(xT[:, c, st * P : (st + 1) * P], tp[:])
                        else:
                            nc.scalar.copy(xT[:, c, st * P : (st + 1) * P], tp[:])

            # ---- phase 2: projections ----
            with (
                tc.tile_pool(name="pjp", bufs=4, space="PSUM") as pjp,
                tc.tile_pool(name="vtp", bufs=1) as vtp,
            ):
                # qT / kT / vT: out[M=128 slot-stacked channels, N=512 seq]
                vT = vtp.tile([P, S], F32R)
                for dst, wofs in ((qT, 0), (kT, P), (vT, 2 * P)):
                    for t in range(NQT):
                        pj = pjp.tile([P, QT_W], F32)
                        for c in range(KC):
                            nc.tensor.matmul(
                                pj[:],
                                w_r[:, c, wofs : wofs + P],
                                xT[:, c, t * QT_W : (t + 1) * QT_W],
                                start=(c == 0),
                                stop=(c == KC - 1),
                            )
                        if t % 2 == 0:
                            nc.vector.tensor_copy(dst[:, t * QT_W : (t + 1) * QT_W], pj[:])
                        else:
                            nc.scalar.copy(dst[:, t * QT_W : (t + 1) * QT_W], pj[:])

                # V natural layout: transpose vT blocks; ones columns for the
                # softmax denominator ride along as channel 64 of each slot.
                ones_c = const.tile([P, 1], F32)
                nc.gpsimd.memset(ones_c[:], 1.0)
                nc.vector.tensor_copy(
                    vA[:, :, 64], ones_c[:, 0:1].broadcast_to([P, NKB])
                )
                nc.vector.tensor_copy(
                    vA[:, :, 129], ones_c[:, 0:1].broadcast_to([P, NKB])
                )
                for kb in range(NKB):
                    tp2 = pjp.tile([P, P], F32R)
                    nc.tensor.transpose(
                        tp2[:], vT[:, kb * P : (kb + 1) * P], ident_r[:]
                    )
                    if kb % 2 == 0:
                        nc.vector.tensor_copy(vA[:, kb, 0:64], tp2[:, 0:64])
                        nc.scalar.copy(vA[:, kb, 65:129], tp2[:, 64:128])
                    else:
                        nc.scalar.copy(vA[:, kb, 0:64], tp2[:, 0:64])
                        nc.vector.tensor_copy(vA[:, kb, 65:129], tp2[:, 64:128])

            # ---- phase 3: attention ----
            with (
                tc.tile_pool(name="scp", bufs=4, space="PSUM") as scp,
                tc.tile_pool(name="ctp", bufs=2, space="PSUM") as ctp,
                tc.tile_pool(name="pt", bufs=6) as pt,
                tc.tile_pool(name="sm", bufs=4) as sm,
            ):
                for t in range(NQT):
                    nkb = 4 * (t + 1)
                    ctx_ps = [ctp.tile([65, QT_W], F32, name=f"ctx{s}") for s in (0, 1)]
                    for kb in range(nkb):
                        r = kb * P - t * QT_W  # diagonal offset
                        r0 = max(0, r)
                        p_tiles = []
                        for slot in (0, 1):
                            sc = scp.tile([P, QT_W], F32)
                            nc.tensor.matmul(
                                sc[:],
                                kT[slot * 64 : slot * 64 + 64, kb * P : (kb + 1) * P],
                                qT[slot * 64 : slot * 64 + 64, t * QT_W : (t + 1) * QT_W],
                                start=True,
                                stop=True,
                            )
                            if r >= 0:
                                nc.vector.tensor_tensor(
                                    sc[:, r : r + P],
                                    sc[:, r : r + P],
                                    mask_s[:],
                                    mybir.AluOpType.add,
                                )
                            p_t = pt.tile([P, QT_W], F32R, name="ptile")
                            nc.scalar.activation(
                                p_t[:, r0:QT_W],
                                sc[:, r0:QT_W],
                                mybir.ActivationFunctionType.Exp,
                                scale=0.125,
                            )
                            p_tiles.append(p_t)
                        for slot in (0, 1):
                            nc.tensor.matmul(
                                ctx_ps[slot][:, r0:QT_W],
                                vA[:, kb, slot * 65 : slot * 65 + 65],
                                p_tiles[slot][:, r0:QT_W],
                                start=(kb == 0),
                                stop=(kb == nkb - 1),
                            )
                    for slot in (0, 1):
                        lr = sm.tile([1, QT_W], F32, name="lrecip")
                        nc.vector.reciprocal(lr[:], ctx_ps[slot][64:65, :])
                        lb = sm.tile([64, QT_W], F32, name="lbcast")
                        nc.gpsimd.partition_broadcast(lb[:], lr[0:1, :])
                        nc.vector.tensor_tensor(
                            cT[slot * 64 : slot * 64 + 64, t * QT_W : (t + 1) * QT_W],
                            ctx_ps[slot][0:64, :],
                            lb[:],
                            mybir.AluOpType.mult,
                        )

            # ---- phase 4: out projection (partial rows of W_out) ----
            with (
                tc.tile_pool(name="opp", bufs=4, space="PSUM") as opp,
                tc.tile_pool(name="ost", bufs=3) as ost,
            ):
                for st in range(S // P):
                    o_stage = ost.tile([P, D], F32)
                    for nch in range(2):
                        po = opp.tile([P, D // 2], F32)
                        nc.tensor.matmul(
                            po[:],
                            cT[:, st * P : (st + 1) * P],
                            wo_r[:, nch * (D // 2) : (nch + 1) * (D // 2)],
                            start=True,
                            stop=True,
                        )
                        if (st + nch) % 2 == 0:
                            nc.vector.tensor_copy(
                                o_stage[:, nch * (D // 2) : (nch + 1) * (D // 2)], po[:]
                            )
                        else:
                            nc.scalar.copy(
                                o_stage[:, nch * (D // 2) : (nch + 1) * (D // 2)], po[:]
                            )
                    nc.sync.dma_start(out_d[st * P : (st + 1) * P, :], o_stage[:])

    nc.compile()
    return nc


def _host_inputs(x, W_query, W_key, W_value, W_out):
    mask = np.where(
        np.arange(P)[:, None] <= np.arange(P)[None, :], 0.0, NEG
    ).astype(np.float32)
    ident = np.eye(P, dtype=np.float32)
    in_maps = []
    for core in range(8):
        ha, hb = SLOTS[core]
        sa, sb = SCALES[core]
        ca, cb = slice(ha * HD, (ha + 1) * HD), slice(hb * HD, (hb + 1) * HD)
        in_maps.append(
            {
                "x": np.ascontiguousarray(x),
                "wq": np.ascontiguousarray(
                    np.concatenate([W_query[:, ca], W_query[:, cb]], axis=1)
                ),
                "wk": np.ascontiguousarray(
                    np.concatenate([W_key[:, ca], W_key[:, cb]], axis=1)
                ),
                "wv": np.ascontiguousarray(
                    np.concatenate([W_value[:, ca], W_value[:, cb]], axis=1)
                ),
                "wo": np.ascontiguousarray(
                    np.concatenate([W_out[ca, :] * sa, W_out[cb, :] * sb], axis=0)
                ),
                "mask": mask,
                "ident": ident,
            }
        )
    return in_maps


def run(x, W_query, W_key, W_value, W_out, b_out, trace=False):
    global _CACHED_NC
    if _CACHED_NC is None:
        _CACHED_NC = build_nc()
    nc = _CACHED_NC
    in_maps = _host_inputs(x, W_query, W_key, W_value, W_out)
    res = run_bass_kernel_spmd(nc, in_maps, core_ids=list(range(8)), trace=trace)
    out = np.zeros((S, D), dtype=np.float32)
    for core in range(8):
        out += res.results[core]["out"]
    out += b_out[None, :].astype(np.float32)
    return out, res


def kernel(x, W_query, W_key, W_value, W_out, b_out):
    x2 = np.asarray(x, dtype=np.float32).reshape(S, D)
    out, _ = run(
        x2,
        np.asarray(W_query, np.float32),
        np.asarray(W_key, np.float32),
        np.asarray(W_value, np.float32),
        np.asarray(W_out, np.float32),
        np.asarray(b_out, np.float32),
    )
    return out.reshape(1, S, D)


# revision 19
# speedup vs baseline: 1.3053x; 1.3053x over previous
"""Causal multi-head attention (B=1, S=4096, D=768, H=12, d_head=64) on 8
Trainium2 NeuronCores.

Sharding: tensor-parallel over heads. 12 heads are mapped onto 16 head-slots
(2 per core); the 4 leftover heads are duplicated onto two slots of the same
core with their W_out rows pre-scaled by 0.5, keeping the SPMD program
uniform across cores. Each core computes Q/K/V projections for its 2 head
slots, causal flash-attention (exp without max-subtraction; softmax
denominator obtained free via an appended ones-column on V), and a partial
row-parallel out-projection. The host sums the 8 partial outputs and adds
b_out (the all-reduce step of the row-parallel out projection).

All matmuls run in float32r with K=128/M=128 (zero-padded where the logical
dims are 64/65) — f32r only hits 1 cycle/row on full 128-wide operands.
"""

import sys

sys.path.insert(0, "/opt/trn_rl_repo")

import numpy as np

import concourse.bass as bass
import concourse.tile as tile
from concourse import bacc, mybir
from concourse.bass_utils import run_bass_kernel_spmd

S = 4096
D = 768
HD = 64
P = 128
KC = D // P  # 6 contraction chunks for the projections
QT_W = 512  # query-tile width (psum free dim)
NQT = S // QT_W  # 8 query tiles
NKB = S // P  # 32 key blocks
NEG = -1e30

F32 = mybir.dt.float32
F32R = mybir.dt.float32r
AF = mybir.ActivationFunctionType

SLOTS = [(0, 1), (2, 3), (4, 5), (6, 7), (8, 8), (9, 9), (10, 10), (11, 11)]
SCALES = [(1.0, 1.0)] * 4 + [(0.5, 0.5)] * 4

_CACHED_NC = None


def build_nc():
    nc = bacc.Bacc("TRN2", target_bir_lowering=False, debug=False, num_devices=8)

    x_d = nc.declare_dram_parameter("x", [S, D], F32, isOutput=False)
    wq_d = nc.declare_dram_parameter("wq", [D, P], F32, isOutput=False)
    wk_d = nc.declare_dram_parameter("wk", [D, P], F32, isOutput=False)
    wv_d = nc.declare_dram_parameter("wv", [D, P], F32, isOutput=False)
    wo_d = nc.declare_dram_parameter("wo", [P, D], F32, isOutput=False)
    mask_d = nc.declare_dram_parameter("mask", [P, P], F32, isOutput=False)
    ident_d = nc.declare_dram_parameter("ident", [P, P], F32, isOutput=False)
    out_d = nc.declare_dram_parameter("out", [S, D], F32, isOutput=True)

    with tile.TileContext(nc) as tc:
        with (
            tc.tile_pool(name="const", bufs=1) as const,
            tc.tile_pool(name="big", bufs=1) as big,
        ):
            # ---- constants ----
            mask_s = const.tile([P, P], F32)
            nc.sync.dma_start(mask_s[:], mask_d[:])
            ident = const.tile([P, P], F32)
            nc.sync.dma_start(ident[:], ident_d[:])
            ident_r = const.tile([P, P], F32R)
            nc.vector.tensor_copy(ident_r[:], ident[:])
            ones_c = const.tile([P, 1], F32)
            nc.gpsimd.memset(ones_c[:], 1.0)
            zero_c = const.tile([P, 1], F32)
            nc.gpsimd.memset(zero_c[:], 0.0)
            wo_r = const.tile([P, D], F32R)

            # qT: slot A rows 0:64, slot B rows 64:128 (no padding needed on
            # the rhs side of the scores matmul). kT per slot, zero-padded on
            # the other 64 rows so the K=128 contraction only picks up its
            # slot. vA: V natural +ones column at 64, zero cols 65:128/slot.
            qT = big.tile([P, S], F32R)
            k2 = [big.tile([P, S], F32R, name=f"k2_{i}") for i in (0, 1)]
            vA = big.tile([P, NKB, 2 * P], F32R)

            nc.vector.tensor_copy(
                k2[0][64:P, :], zero_c[0:64, 0:1].broadcast_to([64, S])
            )
            nc.vector.tensor_copy(
                k2[1][0:64, :], zero_c[0:64, 0:1].broadcast_to([64, S])
            )
            for slot in (0, 1):
                nc.vector.tensor_copy(
                    vA[:, :, slot * P + 65 : slot * P + P],
                    zero_c[:, 0:1].broadcast_to([P, NKB, 63]),
                )
                nc.vector.tensor_copy(
                    vA[:, :, slot * P + 64],
                    ones_c[:, 0:1].broadcast_to([P, NKB]),
                )

            with (
                tc.tile_pool(name="xtp", bufs=1) as xtp,
                tc.tile_pool(name="psA", bufs=2, space="PSUM") as psA,
                tc.tile_pool(name="psB", bufs=4, space="PSUM") as psB,
            ):
                w_r = xtp.tile([P, KC, 3 * P], F32R)
                with tc.tile_pool(name="wst", bufs=1) as wst:
                    w_stage = wst.tile([P, KC, 3 * P], F32)
                    nc.sync.dma_start(
                        w_stage[:, :, 0:P], wq_d.rearrange("(c p) m -> p c m", p=P)
                    )
                    nc.sync.dma_start(
                        w_stage[:, :, P : 2 * P],
                        wk_d.rearrange("(c p) m -> p c m", p=P),
                    )
                    nc.sync.dma_start(
                        w_stage[:, :, 2 * P : 3 * P],
                        wv_d.rearrange("(c p) m -> p c m", p=P),
                    )
                    nc.vector.tensor_copy(w_r[:], w_stage[:])
                    wo_stage = wst.tile([P, D], F32)
                    nc.sync.dma_start(wo_stage[:], wo_d[:])
                    nc.vector.tensor_copy(wo_r[:], wo_stage[:])

                # ---- phase 1: load x, transpose via PE ----
                xT = xtp.tile([P, KC, S], F32R)
                with tc.tile_pool(name="xs", bufs=3) as xs:
                    for st in range(S // P):
                        for half in range(2):
                            x_stage = xs.tile([P, D // 2], F32)
                            nc.sync.dma_start(
                                x_stage[:],
                                x_d[
                                    st * P : (st + 1) * P,
                                    half * (D // 2) : (half + 1) * (D // 2),
                                ],
                            )
                            for ci in range(KC // 2):
                                c = half * (KC // 2) + ci
                                tp = psA.tile([P, P], F32)
                                nc.tensor.transpose(
                                    tp[:], x_stage[:, ci * P : (ci + 1) * P], ident[:]
                                )
                                nc.vector.tensor_copy(
                                    xT[:, c, st * P : (st + 1) * P], tp[:]
                                )

                # ---- phase 2: projections ----
                # psum [128, 512] has slot A rows 0:64, B rows 64:128.
                for t in range(NQT):
                    pj = psB.tile([P, QT_W], F32, name="pjq", tag="pj")
                    for c in range(KC):
                        nc.tensor.matmul(
                            pj[:],
                            w_r[:, c, 0:P],
                            xT[:, c, t * QT_W : (t + 1) * QT_W],
                            start=(c == 0),
                            stop=(c == KC - 1),
                        )
                    nc.vector.tensor_copy(qT[:, t * QT_W : (t + 1) * QT_W], pj[:])
                for t in range(NQT):
                    pj = psB.tile([P, QT_W], F32, name="pjk", tag="pj")
                    for c in range(KC):
                        nc.tensor.matmul(
                            pj[:],
                            w_r[:, c, P : 2 * P],
                            xT[:, c, t * QT_W : (t + 1) * QT_W],
                            start=(c == 0),
                            stop=(c == KC - 1),
                        )
                    nc.vector.tensor_copy(
                        k2[0][0:64, t * QT_W : (t + 1) * QT_W], pj[0:64, :]
                    )
                    nc.vector.tensor_copy(
                        k2[1][64:P, t * QT_W : (t + 1) * QT_W], pj[64:P, :]
                    )

                # V^T then per-block PE transpose into natural layout
                for t in range(NQT):
                    pj = psB.tile([P, QT_W], F32, name="pjv", tag="pj")
                    for c in range(KC):
                        nc.tensor.matmul(
                            pj[:],
                            w_r[:, c, 2 * P : 3 * P],
                            xT[:, c, t * QT_W : (t + 1) * QT_W],
                            start=(c == 0),
                            stop=(c == KC - 1),
                        )
                    vt = xs2 = None
                    vt_t = xtp.tile([P, QT_W], F32R, name="vt_t", tag="vt_t", bufs=1)
                    nc.vector.tensor_copy(vt_t[:], pj[:])
                    for b in range(QT_W // P):
                        kb = t * 4 + b
                        tp2 = psA.tile([P, P], F32R)
                        nc.tensor.transpose(
                            tp2[:], vt_t[:, b * P : (b + 1) * P], ident_r[:]
                        )
                        nc.vector.tensor_copy(vA[:, kb, 0:64], tp2[:, 0:64])
                        nc.vector.tensor_copy(vA[:, kb, P : P + 64], tp2[:, 64:P])

            # ---- phase 3: attention ----
            cT = None
            with tc.tile_pool(name="ctx_sb", bufs=1) as ctx_sb:
              cT = ctx_sb.tile([P, S], F32R)
              with (
                tc.tile_pool(name="scp", bufs=4, space="PSUM") as scp,
                tc.tile_pool(name="ctp", bufs=2, space="PSUM") as ctp,
                tc.tile_pool(name="pt", bufs=6) as pt,
                tc.tile_pool(name="sm", bufs=4) as sm,
              ):
                for t in range(NQT):
                    nkb = 4 * (t + 1)
                    ctx_ps = [
                        ctp.tile([P, QT_W], F32, name=f"ctx{s}") for s in (0, 1)
                    ]
                    for kb in range(nkb):
                        r = kb * P - t * QT_W  # diagonal offset
                        r0 = max(0, r)
                        p_tiles = []
                        for slot in (0, 1):
                            sc = scp.tile([P, QT_W], F32)
                            nc.tensor.matmul(
                                sc[:],
                                k2[slot][:, kb * P : (kb + 1) * P],
                                qT[:, t * QT_W : (t + 1) * QT_W],
                                start=True,
                                stop=True,
                            )
                            if r >= 0:
                                nc.vector.tensor_tensor(
                                    sc[:, r : r + P],
                                    sc[:, r : r + P],
                                    mask_s[:],
                                    mybir.AluOpType.add,
                                )
                            p_t = pt.tile([P, QT_W], F32R, name="ptile")
                            nc.scalar.activation(
                                p_t[:, r0:QT_W],
                                sc[:, r0:QT_W],
                                AF.Exp,
                                scale=0.125,
                            )
                            p_tiles.append(p_t)
                        for slot in (0, 1):
                            nc.tensor.matmul(
                                ctx_ps[slot][:, r0:QT_W],
                                vA[:, kb, slot * P : (slot + 1) * P],
                                p_tiles[slot][:, r0:QT_W],
                                start=(kb == 0),
                                stop=(kb == nkb - 1),
                            )
                    for slot in (0, 1):
                        # 1/l via exp(-log(l)) on ScalarE (DVE reciprocal on a
                        # single partition is ~7 cycles/elem serial)
                        lg = sm.tile([1, QT_W], F32, name="lg")
                        nc.scalar.activation(
                            lg[:], ctx_ps[slot][64:65, :], AF.Ln
                        )
                        lr = sm.tile([1, QT_W], F32, name="lr")
                        nc.scalar.activation(lr[:], lg[:], AF.Exp, scale=-1.0)
                        lb = sm.tile([64, QT_W], F32, name="lb")
                        nc.gpsimd.partition_broadcast(lb[:], lr[0:1, :])
                        nc.vector.tensor_tensor(
                            cT[slot * 64 : slot * 64 + 64, t * QT_W : (t + 1) * QT_W],
                            ctx_ps[slot][0:64, :],
                            lb[:],
                            mybir.AluOpType.mult,
                        )

              # ---- phase 4: out projection (partial rows of W_out) ----
              with (
                tc.tile_pool(name="opp", bufs=4, space="PSUM") as opp,
                tc.tile_pool(name="ost", bufs=3) as ost,
              ):
                for st in range(S // P):
                    o_stage = ost.tile([P, D], F32)
                    for nch in range(2):
                        po = opp.tile([P, D // 2], F32)
                        nc.tensor.matmul(
                            po[:],
                            cT[:, st * P : (st + 1) * P],
                            wo_r[:, nch * (D // 2) : (nch + 1) * (D // 2)],
                            start=True,
                            stop=True,
                        )
                        nc.vector.tensor_copy(
                            o_stage[:, nch * (D // 2) : (nch + 1) * (D // 2)],
                            po[:],
                        )
                    nc.sync.dma_start(out_d[st * P : (st + 1) * P, :], o_stage[:])



    nc.compile()
    return nc


def _host_inputs(x, W_query, W_key, W_value, W_out):
    mask = np.where(
        np.arange(P)[:, None] <= np.arange(P)[None, :], 0.0, NEG
    ).astype(np.float32)
    ident = np.eye(P, dtype=np.float32)
    in_maps = []
    for core in range(8):
        ha, hb = SLOTS[core]
        sa, sb = SCALES[core]
        ca, cb = slice(ha * HD, (ha + 1) * HD), slice(hb * HD, (hb + 1) * HD)
        in_maps.append(
            {
                "x": np.ascontiguousarray(x),
                "wq": np.ascontiguousarray(
                    np.concatenate([W_query[:, ca], W_query[:, cb]], axis=1)
                ),
                "wk": np.ascontiguousarray(
                    np.concatenate([W_key[:, ca], W_key[:, cb]], axis=1)
                ),
                "wv": np.ascontiguousarray(
                    np.concatenate([W_value[:, ca], W_value[:, cb]], axis=1)
                ),
                "wo": np.ascontiguousarray(
                    np.concatenate([W_out[ca, :] * sa, W_out[cb, :] * sb], axis=0)
                ),
                "mask": mask,
                "ident": ident,
            }
        )
    return in_maps


def run(x, W_query, W_key, W_value, W_out, b_out, trace=False):
    global _CACHED_NC
    if _CACHED_NC is None:
        _CACHED_NC = build_nc()
    nc = _CACHED_NC
    in_maps = _host_inputs(x, W_query, W_key, W_value, W_out)
    res = run_bass_kernel_spmd(nc, in_maps, core_ids=list(range(8)), trace=trace)
    out = np.zeros((S, D), dtype=np.float32)
    for core in range(8):
        out += res.results[core]["out"]
    out += b_out[None, :].astype(np.float32)
    return out, res


def kernel(x, W_query, W_key, W_value, W_out, b_out):
    x2 = np.asarray(x, dtype=np.float32).reshape(S, D)
    out, _ = run(
        x2,
        np.asarray(W_query, np.float32),
        np.asarray(W_key, np.float32),
        np.asarray(W_value, np.float32),
        np.asarray(W_out, np.float32),
        np.asarray(b_out, np.float32),
    )
    return out.reshape(1, S, D)


# revision 26
# speedup vs baseline: 1.4456x; 1.1075x over previous
"""Causal multi-head attention (B=1, S=4096, D=768, H=12, d_head=64) on 8
Trainium2 NeuronCores.

Sharding: tensor-parallel over heads. 12 heads are mapped onto 16 head-slots
(2 per core); the 4 leftover heads are duplicated onto two slots of the same
core with their W_out rows pre-scaled by 0.5, keeping the SPMD program
uniform across cores. Each core computes Q/K/V projections for its 2 head
slots, causal flash-attention (exp without max-subtraction; softmax
denominator obtained free via an appended ones-column on V), and a partial
row-parallel out-projection. The host sums the 8 partial outputs and adds
b_out (the all-reduce step of the row-parallel out projection).

All matmuls run in float32r with K=128/M=128 (zero-padded where the logical
dims are 64/65) — f32r only hits 1 cycle/row on full 128-wide operands.
"""

import sys

sys.path.insert(0, "/opt/trn_rl_repo")

import numpy as np

import concourse.bass as bass
import concourse.tile as tile
from concourse import bacc, mybir
from concourse.bass_utils import run_bass_kernel_spmd

S = 4096
D = 768
HD = 64
P = 128
KC = D // P  # 6 contraction chunks for the projections
QT_W = 512  # query-tile width (psum free dim)
NQT = S // QT_W  # 8 query tiles
NKB = S // P  # 32 key blocks
NEG = -1e30

F32 = mybir.dt.float32
F32R = mybir.dt.float32r
AF = mybir.ActivationFunctionType

SLOTS = [(0, 1), (2, 3), (4, 5), (6, 7), (8, 8), (9, 9), (10, 10), (11, 11)]
SCALES = [(1.0, 1.0)] * 4 + [(0.5, 0.5)] * 4

_CACHED_NC = None


def build_nc():
    nc = bacc.Bacc("TRN2", target_bir_lowering=False, debug=False, num_devices=8)

    x_d = nc.declare_dram_parameter("x", [S, D], F32, isOutput=False)
    wq_d = nc.declare_dram_parameter("wq", [D, P], F32, isOutput=False)
    wk_d = nc.declare_dram_parameter("wk", [D, P], F32, isOutput=False)
    wv_d = nc.declare_dram_parameter("wv", [D, P], F32, isOutput=False)
    wo_d = nc.declare_dram_parameter("wo", [P, D], F32, isOutput=False)
    mask_d = nc.declare_dram_parameter("mask", [P, P], F32, isOutput=False)
    ident_d = nc.declare_dram_parameter("ident", [P, P], F32, isOutput=False)
    out_d = nc.declare_dram_parameter("out", [S, D], F32, isOutput=True)

    with tile.TileContext(nc) as tc:
        with (
            tc.tile_pool(name="const", bufs=1) as const,
            tc.tile_pool(name="big", bufs=1) as big,
        ):
            # ---- constants ----
            mask_s = const.tile([P, P], F32)
            nc.sync.dma_start(mask_s[:], mask_d[:])
            ident = const.tile([P, P], F32)
            nc.sync.dma_start(ident[:], ident_d[:])
            ident_r = const.tile([P, P], F32R)
            nc.vector.tensor_copy(ident_r[:], ident[:])
            wpool = const  # warmup matmuls: get the PE HAM to 2.4 GHz while
            # the x DMA streams in
            ones_c = const.tile([P, 1], F32)
            nc.gpsimd.memset(ones_c[:], 1.0)
            zero_c = const.tile([P, 1], F32)
            nc.gpsimd.memset(zero_c[:], 0.0)
            wo_r = const.tile([P, D], F32R)

            # qT: slot A rows 0:64, slot B rows 64:128 (no padding needed on
            # the rhs side of the scores matmul). kT per slot, zero-padded on
            # the other 64 rows so the K=128 contraction only picks up its
            # slot. vA: V natural +ones column at 64, zero cols 65:128/slot.
            qT = big.tile([P, S], F32R)
            k2 = [big.tile([P, S], F32R, name=f"k2_{i}") for i in (0, 1)]
            vA = big.tile([P, NKB, 2 * P], F32R)

            nc.vector.tensor_copy(
                k2[0][64:P, :], zero_c[0:64, 0:1].broadcast_to([64, S])
            )
            nc.vector.tensor_copy(
                k2[1][0:64, :], zero_c[0:64, 0:1].broadcast_to([64, S])
            )
            for slot in (0, 1):
                nc.vector.tensor_copy(
                    vA[:, :, slot * P + 65 : slot * P + P],
                    zero_c[:, 0:1].broadcast_to([P, NKB, 63]),
                )
                nc.vector.tensor_copy(
                    vA[:, :, slot * P + 64],
                    ones_c[:, 0:1].broadcast_to([P, NKB]),
                )

            with (
                tc.tile_pool(name="xtp", bufs=1) as xtp,
                tc.tile_pool(name="psA", bufs=2, space="PSUM") as psA,
                tc.tile_pool(name="psB", bufs=4, space="PSUM") as psB,
            ):
                for wi in range(48):
                    wps = psA.tile([P, P], F32, name="tp", tag="tp")
                    nc.tensor.matmul(
                        wps[:], ident_r[:], ident_r[:], start=True, stop=True
                    )
                w_r = xtp.tile([P, KC, 3 * P], F32R)
                with tc.tile_pool(name="wst", bufs=1) as wst:
                    w_stage = wst.tile([P, KC, 3 * P], F32)
                    nc.sync.dma_start(
                        w_stage[:, :, 0:P], wq_d.rearrange("(c p) m -> p c m", p=P)
                    )
                    nc.sync.dma_start(
                        w_stage[:, :, P : 2 * P],
                        wk_d.rearrange("(c p) m -> p c m", p=P),
                    )
                    nc.sync.dma_start(
                        w_stage[:, :, 2 * P : 3 * P],
                        wv_d.rearrange("(c p) m -> p c m", p=P),
                    )
                    nc.vector.tensor_copy(w_r[:], w_stage[:])
                    wo_stage = wst.tile([P, D], F32)
                    nc.sync.dma_start(wo_stage[:], wo_d[:])
                    nc.vector.tensor_copy(wo_r[:], wo_stage[:])

                # ---- phases 1+2 interleaved: per q-tile group, DMA x,
                # transpose via PE, then Q/K/V projections for that group ----
                xT = xtp.tile([P, KC, S], F32R)
                with tc.tile_pool(name="xs", bufs=4) as xs:
                    for t in range(NQT):
                        for sti in range(4):
                            st = t * 4 + sti
                            for half in range(2):
                                x_stage = xs.tile([P, D // 2], F32)
                                nc.sync.dma_start(
                                    x_stage[:],
                                    x_d[
                                        st * P : (st + 1) * P,
                                        half * (D // 2) : (half + 1) * (D // 2),
                                    ],
                                )
                                for ci in range(KC // 2):
                                    c = half * (KC // 2) + ci
                                    tp = psA.tile([P, P], F32)
                                    nc.tensor.transpose(
                                        tp[:],
                                        x_stage[:, ci * P : (ci + 1) * P],
                                        ident[:],
                                    )
                                    nc.vector.tensor_copy(
                                        xT[:, c, st * P : (st + 1) * P], tp[:]
                                    )
                        # Q projection for this q-tile group
                        pj = psB.tile([P, QT_W], F32, name="pjq", tag="pj")
                        for c in range(KC):
                            nc.tensor.matmul(
                                pj[:],
                                w_r[:, c, 0:P],
                                xT[:, c, t * QT_W : (t + 1) * QT_W],
                                start=(c == 0),
                                stop=(c == KC - 1),
                            )
                        nc.vector.tensor_copy(qT[:, t * QT_W : (t + 1) * QT_W], pj[:])
                        # K projection
                        pj = psB.tile([P, QT_W], F32, name="pjk", tag="pj")
                        for c in range(KC):
                            nc.tensor.matmul(
                                pj[:],
                                w_r[:, c, P : 2 * P],
                                xT[:, c, t * QT_W : (t + 1) * QT_W],
                                start=(c == 0),
                                stop=(c == KC - 1),
                            )
                        nc.vector.tensor_copy(
                            k2[0][0:64, t * QT_W : (t + 1) * QT_W], pj[0:64, :]
                        )
                        nc.vector.tensor_copy(
                            k2[1][64:P, t * QT_W : (t + 1) * QT_W], pj[64:P, :]
                        )
                        # V projection + transpose to natural layout
                        pj = psB.tile([P, QT_W], F32, name="pjv", tag="pj")
                        for c in range(KC):
                            nc.tensor.matmul(
                                pj[:],
                                w_r[:, c, 2 * P : 3 * P],
                                xT[:, c, t * QT_W : (t + 1) * QT_W],
                                start=(c == 0),
                                stop=(c == KC - 1),
                            )
                        vt_t = xtp.tile(
                            [P, QT_W], F32R, name="vt_t", tag="vt_t", bufs=2
                        )
                        nc.vector.tensor_copy(vt_t[:], pj[:])
                        for b in range(QT_W // P):
                            kb = t * 4 + b
                            tp2 = psA.tile([P, P], F32R)
                            nc.tensor.transpose(
                                tp2[:], vt_t[:, b * P : (b + 1) * P], ident_r[:]
                            )
                            nc.vector.tensor_copy(vA[:, kb, 0:64], tp2[:, 0:64])
                            nc.vector.tensor_copy(
                                vA[:, kb, P : P + 64], tp2[:, 64:P]
                            )

            # ---- phase 3: attention ----
            cT = None
            with tc.tile_pool(name="ctx_sb", bufs=1) as ctx_sb:
              cT = ctx_sb.tile([P, S], F32R)
              with (
                tc.tile_pool(name="scp", bufs=4, space="PSUM") as scp,
                tc.tile_pool(name="ctp", bufs=2, space="PSUM") as ctp,
                tc.tile_pool(name="pt", bufs=6) as pt,
                tc.tile_pool(name="sm", bufs=4) as sm,
              ):
                for t in range(NQT):
                    nkb = 4 * (t + 1)
                    ctx_ps = [
                        ctp.tile([P, QT_W], F32, name=f"ctx{s}") for s in (0, 1)
                    ]
                    for kb in range(nkb):
                        r = kb * P - t * QT_W  # diagonal offset
                        r0 = max(0, r)
                        p_tiles = []
                        for slot in (0, 1):
                            sc = scp.tile([P, QT_W], F32)
                            nc.tensor.matmul(
                                sc[:],
                                k2[slot][:, kb * P : (kb + 1) * P],
                                qT[:, t * QT_W : (t + 1) * QT_W],
                                start=True,
                                stop=True,
                            )
                            if r >= 0:
                                nc.vector.tensor_tensor(
                                    sc[:, r : r + P],
                                    sc[:, r : r + P],
                                    mask_s[:],
                                    mybir.AluOpType.add,
                                )
                            p_t = pt.tile([P, QT_W], F32R, name="ptile")
                            nc.scalar.activation(
                                p_t[:, r0:QT_W],
                                sc[:, r0:QT_W],
                                AF.Exp,
                                scale=0.125,
                            )
                            p_tiles.append(p_t)
                        for slot in (0, 1):
                            nc.tensor.matmul(
                                ctx_ps[slot][:, r0:QT_W],
                                vA[:, kb, slot * P : (slot + 1) * P],
                                p_tiles[slot][:, r0:QT_W],
                                start=(kb == 0),
                                stop=(kb == nkb - 1),
                            )
                    # both slots' softmax denominators in one DVE
                    # reciprocal (serial per-lane op: batch to amortize)
                    for slot in (0, 1):
                        lr = sm.tile([1, QT_W], F32, name="lrecip")
                        nc.vector.reciprocal(lr[:], ctx_ps[slot][64:65, :])
                        lb = sm.tile([64, QT_W], F32, name="lb")
                        nc.gpsimd.partition_broadcast(lb[:], lr[0:1, :])
                        nc.vector.tensor_tensor(
                            cT[slot * 64 : slot * 64 + 64, t * QT_W : (t + 1) * QT_W],
                            ctx_ps[slot][0:64, :],
                            lb[:],
                            mybir.AluOpType.mult,
                        )

              # ---- phase 4: out projection (partial rows of W_out) ----
              with (
                tc.tile_pool(name="opp", bufs=4, space="PSUM") as opp,
                tc.tile_pool(name="ost", bufs=3) as ost,
              ):
                for st in range(S // P):
                    o_stage = ost.tile([P, D], F32)
                    for nch in range(2):
                        po = opp.tile([P, D // 2], F32)
                        nc.tensor.matmul(
                            po[:],
                            cT[:, st * P : (st + 1) * P],
                            wo_r[:, nch * (D // 2) : (nch + 1) * (D // 2)],
                            start=True,
                            stop=True,
                        )
                        nc.vector.tensor_copy(
                            o_stage[:, nch * (D // 2) : (nch + 1) * (D // 2)],
                            po[:],
                        )
                    nc.sync.dma_start(out_d[st * P : (st + 1) * P, :], o_stage[:])



    nc.compile()
    return nc


def _host_inputs(x, W_query, W_key, W_value, W_out):
    mask = np.where(
        np.arange(P)[:, None] <= np.arange(P)[None, :], 0.0, NEG
    ).astype(np.float32)
    ident = np.eye(P, dtype=np.float32)
    in_maps = []
    for core in range(8):
        ha, hb = SLOTS[core]
        sa, sb = SCALES[core]
        ca, cb = slice(ha * HD, (ha + 1) * HD), slice(hb * HD, (hb + 1) * HD)
        in_maps.append(
            {
                "x": np.ascontiguousarray(x),
                "wq": np.ascontiguousarray(
                    np.concatenate([W_query[:, ca], W_query[:, cb]], axis=1)
                ),
                "wk": np.ascontiguousarray(
                    np.concatenate([W_key[:, ca], W_key[:, cb]], axis=1)
                ),
                "wv": np.ascontiguousarray(
                    np.concatenate([W_value[:, ca], W_value[:, cb]], axis=1)
                ),
                "wo": np.ascontiguousarray(
                    np.concatenate([W_out[ca, :] * sa, W_out[cb, :] * sb], axis=0)
                ),
                "mask": mask,
                "ident": ident,
            }
        )
    return in_maps


def run(x, W_query, W_key, W_value, W_out, b_out, trace=False):
    global _CACHED_NC
    if _CACHED_NC is None:
        _CACHED_NC = build_nc()
    nc = _CACHED_NC
    in_maps = _host_inputs(x, W_query, W_key, W_value, W_out)
    res = run_bass_kernel_spmd(nc, in_maps, core_ids=list(range(8)), trace=trace)
    out = np.zeros((S, D), dtype=np.float32)
    for core in range(8):
        out += res.results[core]["out"]
    out += b_out[None, :].astype(np.float32)
    return out, res


def kernel(x, W_query, W_key, W_value, W_out, b_out):
    x2 = np.asarray(x, dtype=np.float32).reshape(S, D)
    out, _ = run(
        x2,
        np.asarray(W_query, np.float32),
        np.asarray(W_key, np.float32),
        np.asarray(W_value, np.float32),
        np.asarray(W_out, np.float32),
        np.asarray(b_out, np.float32),
    )
    return out.reshape(1, S, D)


# revision 30
# speedup vs baseline: 1.4609x; 1.0106x over previous
"""Causal multi-head attention (B=1, S=4096, D=768, H=12, d_head=64) on 8
Trainium2 NeuronCores.

Sharding: tensor-parallel over heads. 12 heads are mapped onto 16 head-slots
(2 per core); the 4 leftover heads are duplicated onto two slots of the same
core with their W_out rows pre-scaled by 0.5, keeping the SPMD program
uniform across cores. Each core computes Q/K/V projections for its 2 head
slots, causal flash-attention (exp without max-subtraction; softmax
denominator obtained free via an appended ones-column on V), and a partial
row-parallel out-projection. The host sums the 8 partial outputs and adds
b_out (the all-reduce step of the row-parallel out projection).

All matmuls run in float32r with K=128/M=128 (zero-padded where the logical
dims are 64/65) — f32r only hits 1 cycle/row on full 128-wide operands.
"""

import sys

sys.path.insert(0, "/opt/trn_rl_repo")

import numpy as np

import concourse.bass as bass
import concourse.tile as tile
from concourse import bacc, mybir
from concourse.bass_utils import run_bass_kernel_spmd

S = 4096
D = 768
HD = 64
P = 128
KC = D // P  # 6 contraction chunks for the projections
QT_W = 512  # query-tile width (psum free dim)
NQT = S // QT_W  # 8 query tiles
NKB = S // P  # 32 key blocks
NEG = -1e30

F32 = mybir.dt.float32
F32R = mybir.dt.float32r
AF = mybir.ActivationFunctionType

SLOTS = [(0, 1), (2, 3), (4, 5), (6, 7), (8, 8), (9, 9), (10, 10), (11, 11)]
SCALES = [(1.0, 1.0)] * 4 + [(0.5, 0.5)] * 4

_CACHED_NC = None


def build_nc():
    nc = bacc.Bacc("TRN2", target_bir_lowering=False, debug=False, num_devices=8)

    x_d = nc.declare_dram_parameter("x", [S, D], F32, isOutput=False)
    wq_d = nc.declare_dram_parameter("wq", [D, P], F32, isOutput=False)
    wk_d = nc.declare_dram_parameter("wk", [D, P], F32, isOutput=False)
    wv_d = nc.declare_dram_parameter("wv", [D, P], F32, isOutput=False)
    wo_d = nc.declare_dram_parameter("wo", [P, D], F32, isOutput=False)
    mask_d = nc.declare_dram_parameter("mask", [P, P], F32, isOutput=False)
    ident_d = nc.declare_dram_parameter("ident", [P, P], F32, isOutput=False)
    out_d = nc.declare_dram_parameter("out", [S, D], F32, isOutput=True)

    with tile.TileContext(nc) as tc:
        with (
            tc.tile_pool(name="const", bufs=1) as const,
            tc.tile_pool(name="big", bufs=1) as big,
        ):
            # ---- constants ----
            mask_s = const.tile([P, P], F32)
            nc.sync.dma_start(mask_s[:], mask_d[:])
            ident = const.tile([P, P], F32)
            nc.sync.dma_start(ident[:], ident_d[:])
            ident_r = const.tile([P, P], F32R)
            nc.vector.tensor_copy(ident_r[:], ident[:])
            wpool = const  # warmup matmuls: get the PE HAM to 2.4 GHz while
            # the x DMA streams in
            ones_c = const.tile([P, 1], F32)
            nc.gpsimd.memset(ones_c[:], 1.0)
            zero_c = const.tile([P, 1], F32)
            nc.gpsimd.memset(zero_c[:], 0.0)
            wo_r = const.tile([P, D], F32R)

            # qT: slot A rows 0:64, slot B rows 64:128 (no padding needed on
            # the rhs side of the scores matmul). kT per slot, zero-padded on
            # the other 64 rows so the K=128 contraction only picks up its
            # slot. vA: V natural +ones column at 64, zero cols 65:128/slot.
            qT = big.tile([P, S], F32R)
            k2 = [big.tile([P, S], F32R, name=f"k2_{i}") for i in (0, 1)]
            vA = big.tile([P, NKB, 2 * P], F32R)

            nc.vector.tensor_copy(
                k2[0][64:P, :], zero_c[0:64, 0:1].broadcast_to([64, S])
            )
            nc.vector.tensor_copy(
                k2[1][0:64, :], zero_c[0:64, 0:1].broadcast_to([64, S])
            )
            for slot in (0, 1):
                nc.vector.tensor_copy(
                    vA[:, :, slot * P + 65 : slot * P + P],
                    zero_c[:, 0:1].broadcast_to([P, NKB, 63]),
                )
                nc.vector.tensor_copy(
                    vA[:, :, slot * P + 64],
                    ones_c[:, 0:1].broadcast_to([P, NKB]),
                )

            with (
                tc.tile_pool(name="xtp", bufs=1) as xtp,
                tc.tile_pool(name="psA", bufs=2, space="PSUM") as psA,
                tc.tile_pool(name="psB", bufs=4, space="PSUM") as psB,
            ):
                for wi in range(48):
                    wps = psA.tile([P, P], F32, name="tp", tag="tp")
                    nc.tensor.matmul(
                        wps[:], ident_r[:], ident_r[:], start=True, stop=True
                    )
                w_r = xtp.tile([P, KC, 3 * P], F32R)
                with tc.tile_pool(name="wst", bufs=1) as wst:
                    w_stage = wst.tile([P, KC, 3 * P], F32)
                    nc.sync.dma_start(
                        w_stage[:, :, 0:P], wq_d.rearrange("(c p) m -> p c m", p=P)
                    )
                    nc.sync.dma_start(
                        w_stage[:, :, P : 2 * P],
                        wk_d.rearrange("(c p) m -> p c m", p=P),
                    )
                    nc.sync.dma_start(
                        w_stage[:, :, 2 * P : 3 * P],
                        wv_d.rearrange("(c p) m -> p c m", p=P),
                    )
                    nc.vector.tensor_copy(w_r[:], w_stage[:])
                    wo_stage = wst.tile([P, D], F32)
                    nc.sync.dma_start(wo_stage[:], wo_d[:])
                    nc.vector.tensor_copy(wo_r[:], wo_stage[:])

                # ---- phases 1+2 interleaved: per q-tile group, DMA x,
                # transpose via PE, then Q/K/V projections for that group ----
                xT = xtp.tile([P, KC, S], F32R)
                with tc.tile_pool(name="xs", bufs=4) as xs:
                    for t in range(NQT):
                        for sti in range(4):
                            st = t * 4 + sti
                            for half in range(2):
                                x_stage = xs.tile([P, D // 2], F32)
                                nc.sync.dma_start(
                                    x_stage[:],
                                    x_d[
                                        st * P : (st + 1) * P,
                                        half * (D // 2) : (half + 1) * (D // 2),
                                    ],
                                )
                                for ci in range(KC // 2):
                                    c = half * (KC // 2) + ci
                                    tp = psA.tile([P, P], F32)
                                    nc.tensor.transpose(
                                        tp[:],
                                        x_stage[:, ci * P : (ci + 1) * P],
                                        ident[:],
                                    )
                                    nc.vector.tensor_copy(
                                        xT[:, c, st * P : (st + 1) * P], tp[:]
                                    )
                        # Q projection for this q-tile group
                        pj = psB.tile([P, QT_W], F32, name="pjq", tag="pj")
                        for c in range(KC):
                            nc.tensor.matmul(
                                pj[:],
                                w_r[:, c, 0:P],
                                xT[:, c, t * QT_W : (t + 1) * QT_W],
                                start=(c == 0),
                                stop=(c == KC - 1),
                            )
                        nc.vector.tensor_copy(qT[:, t * QT_W : (t + 1) * QT_W], pj[:])
                        # K projection
                        pj = psB.tile([P, QT_W], F32, name="pjk", tag="pj")
                        for c in range(KC):
                            nc.tensor.matmul(
                                pj[:],
                                w_r[:, c, P : 2 * P],
                                xT[:, c, t * QT_W : (t + 1) * QT_W],
                                start=(c == 0),
                                stop=(c == KC - 1),
                            )
                        nc.vector.tensor_copy(
                            k2[0][0:64, t * QT_W : (t + 1) * QT_W], pj[0:64, :]
                        )
                        nc.vector.tensor_copy(
                            k2[1][64:P, t * QT_W : (t + 1) * QT_W], pj[64:P, :]
                        )
                        # V projection + transpose to natural layout
                        pj = psB.tile([P, QT_W], F32, name="pjv", tag="pj")
                        for c in range(KC):
                            nc.tensor.matmul(
                                pj[:],
                                w_r[:, c, 2 * P : 3 * P],
                                xT[:, c, t * QT_W : (t + 1) * QT_W],
                                start=(c == 0),
                                stop=(c == KC - 1),
                            )
                        vt_t = xtp.tile(
                            [P, QT_W], F32R, name="vt_t", tag="vt_t", bufs=2
                        )
                        nc.vector.tensor_copy(vt_t[:], pj[:])
                        for b in range(QT_W // P):
                            kb = t * 4 + b
                            tp2 = psA.tile([P, P], F32R)
                            nc.tensor.transpose(
                                tp2[:], vt_t[:, b * P : (b + 1) * P], ident_r[:]
                            )
                            nc.vector.tensor_copy(vA[:, kb, 0:64], tp2[:, 0:64])
                            nc.vector.tensor_copy(
                                vA[:, kb, P : P + 64], tp2[:, 64:P]
                            )

            # ---- phase 3: attention ----
            cT = None
            with tc.tile_pool(name="ctx_sb", bufs=1) as ctx_sb:
              cT = ctx_sb.tile([P, S], F32R)
              with (
                tc.tile_pool(name="scp", bufs=4, space="PSUM") as scp,
                tc.tile_pool(name="ctp", bufs=2, space="PSUM") as ctp,
                tc.tile_pool(name="pt", bufs=8) as pt,
                tc.tile_pool(name="sm", bufs=4) as sm,
              ):
                for t in range(NQT):
                    nkb = 4 * (t + 1)
                    ctx_ps = [
                        ctp.tile([P, QT_W], F32, name=f"ctx{s}", tag=f"ctx{s}")
                        for s in (0, 1)
                    ]
                    for kb in range(nkb):
                        r = kb * P - t * QT_W  # diagonal offset
                        r0 = max(0, r)
                        p_tiles = []
                        for slot in (0, 1):
                            sc = scp.tile([P, QT_W], F32, name="sc", tag="sc")
                            nc.tensor.matmul(
                                sc[:],
                                k2[slot][:, kb * P : (kb + 1) * P],
                                qT[:, t * QT_W : (t + 1) * QT_W],
                                start=True,
                                stop=True,
                            )
                            if r >= 0:
                                nc.vector.tensor_tensor(
                                    sc[:, r : r + P],
                                    sc[:, r : r + P],
                                    mask_s[:],
                                    mybir.AluOpType.add,
                                )
                            p_t = pt.tile([P, QT_W], F32R, name="ptile")
                            nc.scalar.activation(
                                p_t[:, r0:QT_W],
                                sc[:, r0:QT_W],
                                AF.Exp,
                                scale=0.125,
                            )
                            p_tiles.append(p_t)
                        for slot in (0, 1):
                            nc.tensor.matmul(
                                ctx_ps[slot][:, r0:QT_W],
                                vA[:, kb, slot * P : (slot + 1) * P],
                                p_tiles[slot][:, r0:QT_W],
                                start=(kb == 0),
                                stop=(kb == nkb - 1),
                            )
                    for slot in (0, 1):
                        lr = sm.tile([1, QT_W], F32, name="lrecip")
                        nc.vector.reciprocal(lr[:], ctx_ps[slot][64:65, :])
                        lb = sm.tile([64, QT_W], F32, name="lb")
                        nc.gpsimd.partition_broadcast(lb[:], lr[0:1, :])
                        nc.vector.tensor_tensor(
                            cT[slot * 64 : slot * 64 + 64, t * QT_W : (t + 1) * QT_W],
                            ctx_ps[slot][0:64, :],
                            lb[:],
                            mybir.AluOpType.mult,
                        )

                # out projection fused in the same scope so the PE stays warm
                for st in range(S // P):
                    o_stage = sm.tile([P, D], F32, name="o_stage", bufs=3)
                    for nch in range(2):
                        po = scp.tile([P, QT_W], F32, name="sc", tag="sc")
                        nc.tensor.matmul(
                            po[:, : D // 2],
                            cT[:, st * P : (st + 1) * P],
                            wo_r[:, nch * (D // 2) : (nch + 1) * (D // 2)],
                            start=True,
                            stop=True,
                        )
                        nc.vector.tensor_copy(
                            o_stage[:, nch * (D // 2) : (nch + 1) * (D // 2)],
                            po[:, : D // 2],
                        )
                    nc.sync.dma_start(out_d[st * P : (st + 1) * P, :], o_stage[:])

    nc.compile()
    return nc


def _host_inputs(x, W_query, W_key, W_value, W_out):
    mask = np.where(
        np.arange(P)[:, None] <= np.arange(P)[None, :], 0.0, NEG
    ).astype(np.float32)
    ident = np.eye(P, dtype=np.float32)
    in_maps = []
    for core in range(8):
        ha, hb = SLOTS[core]
        sa, sb = SCALES[core]
        ca, cb = slice(ha * HD, (ha + 1) * HD), slice(hb * HD, (hb + 1) * HD)
        in_maps.append(
            {
                "x": np.ascontiguousarray(x),
                "wq": np.ascontiguousarray(
                    np.concatenate([W_query[:, ca], W_query[:, cb]], axis=1)
                ),
                "wk": np.ascontiguousarray(
                    np.concatenate([W_key[:, ca], W_key[:, cb]], axis=1)
                ),
                "wv": np.ascontiguousarray(
                    np.concatenate([W_value[:, ca], W_value[:, cb]], axis=1)
                ),
                "wo": np.ascontiguousarray(
                    np.concatenate([W_out[ca, :] * sa, W_out[cb, :] * sb], axis=0)
                ),
                "mask": mask,
                "ident": ident,
            }
        )
    return in_maps


def run(x, W_query, W_key, W_value, W_out, b_out, trace=False):
    global _CACHED_NC
    if _CACHED_NC is None:
        _CACHED_NC = build_nc()
    nc = _CACHED_NC
    in_maps = _host_inputs(x, W_query, W_key, W_value, W_out)
    res = run_bass_kernel_spmd(nc, in_maps, core_ids=list(range(8)), trace=trace)
    out = np.zeros((S, D), dtype=np.float32)
    for core in range(8):
        out += res.results[core]["out"]
    out += b_out[None, :].astype(np.float32)
    return out, res


def kernel(x, W_query, W_key, W_value, W_out, b_out):
    x2 = np.asarray(x, dtype=np.float32).reshape(S, D)
    out, _ = run(
        x2,
        np.asarray(W_query, np.float32),
        np.asarray(W_key, np.float32),
        np.asarray(W_value, np.float32),
        np.asarray(W_out, np.float32),
        np.asarray(b_out, np.float32),
    )
    return out.reshape(1, S, D)
